# revision 29
# baseline (speedup 1.0000x reference)
"""Multi-head self-attention (B=2, S=2048, D=1024, H=16) on 8 TRN2 NeuronCores.

Sharding: batch*heads tensor-parallel. Each core owns 2 heads (both batches):
QKV projection for its heads only (W_qkv output-dim sharded), full attention
for its 2x2 (batch, head) pairs, partial output projection (W_out input-dim
sharded). The 8 partial outputs are summed on the host (the "all-reduce").

v3 schedule — ACT(exp)-saturated single pipeline, no serial head phase:
  - Trace analysis of v2 showed ACT-exp is the long pole (1.22us per ki,
    128 ki = 156us floor) but ACT idled for the first ~45us (serial batch-0
    QKV head) and much of the 20us tail. v3 computes ONLY chunk-0 QKV
    eagerly (~4us of PE), then enters the attention slot loop; every other
    piece of QKV (b0 ch1-3, all of b1) plus norm/output-projection work is
    a deadline-tagged closure popped into PE slack between per-ki attention
    steps. First exp fires ~7us in; ACT stays ~95%+ busy to the end.
  - Input DMA spread across sync/vector/gpsimd queues (nothing on the
    scalar queue: ACT's instruction stream is exp-only). x chunk 0 is
    split in halves on the vector queue; W ships as three per-g tensors so
    the k/q parts land in ~1.5us on sync.
  - Attention per quarter unchanged from v2: 512-token query quarters,
    double-buffered [128,1024] score PSUM ring, one exp ACTIVATE per ki
    covering both heads, AV at skew-2, ones-column denominators,
    reciprocal via DMA round trip.
  - Worklist closures are grouped in series that share a PSUM wk-ring
    tile; series are atomic wrt other wk-allocating closures (two queues:
    QKV series + norm/proj series, interleaved only at series boundaries).
Matmul dtypes: bf16 throughout (fp32 only for the K=1 denominator
broadcast matmul and PSUM accumulation).
"""

import math
import sys
from collections import deque

for _p in ("/opt/trn_rl_repo", "/root/.axon_site/_ro/trn_rl_repo"):
    if _p not in sys.path:
        sys.path.insert(0, _p)

from contextlib import ExitStack

import numpy as np

import concourse.bacc as bacc
import concourse.bass as bass
import concourse.mybir as mybir
import concourse.tile as tile
from concourse.bass_utils import run_bass_kernel_spmd

F32 = mybir.dt.float32
BF16 = mybir.dt.bfloat16

B, S, D, H = 2, 2048, 1024, 16
HD = D // H  # 64
T = B * S  # 4096 tokens
SCALE = HD**-0.5
N_CORES = 8
HEADS_PER_CORE = H // N_CORES  # 2
NQ = 4  # query quarters per batch (512 tokens each)
KI = 16  # key chunks of 128
QW = S // NQ  # 512

EXP = mybir.ActivationFunctionType.Exp
DEBUG_DENOM = False


class Worklist:
    """Two FIFO queues of closure-series popped into attention PE slack.

    Each item is a series: a list of closures sharing one PSUM wk-ring
    allocation, with a ready slot (not popped before) and a deadline slot
    (fully popped before that attention iteration runs). Series from the
    two queues never interleave mid-series (the wk ring has only 2
    buffers). Queue A holds QKV work (hard deadlines), queue B norm/proj
    work (ready-gated, soft deadlines).
    """

    def __init__(self):
        self.qa = deque()
        self.qb = deque()
        self.open_q = None  # queue whose front series is partially popped

    def add_a(self, fns, ready=0, deadline=10**9):
        self.qa.append([ready, deadline, deque(fns)])

    def add_b(self, fns, ready=0):
        self.qb.append([ready, 10**9, deque(fns)])

    def _pop_one(self, q):
        item = q[0]
        item[2].popleft()()
        if not item[2]:
            q.popleft()
            self.open_q = None
        else:
            self.open_q = q

    def pop_for_slot(self, s, horizon=128):
        budget = 4
        qb_cap = 2  # norm/proj closures are DVE-heavy; avoid flooding DVE
        popped = 0
        # an open series blocks everything else: finish it first
        while self.open_q is not None and popped < budget:
            self._pop_one(self.open_q)
            popped += 1
        if self.open_q is not None:
            return
        # emergency: every series due before the next iteration
        while self.qa and self.qa[0][1] <= s + 1:
            self._pop_one(self.qa)
            popped += 1
            while self.open_q is not None:
                self._pop_one(self.open_q)
                popped += 1
        # paced pops, alternating between the queues
        rem = sum(len(i[2]) for i in self.qa) + sum(len(i[2]) for i in self.qb)
        pace = min(max(0, budget - popped), math.ceil(rem / max(1, horizon - s)))
        n = 0
        nb = 0
        toggle = s & 1
        while n < pace:
            qs = [self.qa, self.qb] if toggle == 0 else [self.qb, self.qa]
            if nb >= qb_cap:
                qs = [self.qa]
            q = next((q for q in qs if q and q[0][0] <= s), None)
            if q is None:
                break
            if q is self.qb:
                nb += 1
            self._pop_one(q)
            n += 1
            while self.open_q is not None and popped + n < budget + 4:
                self._pop_one(self.open_q)
                n += 1
            if self.open_q is not None:
                break
            toggle ^= 1

    def drain(self):
        for q in (self.qa, self.qb):
            while q:
                self._pop_one(q)


def build_kernel() -> bacc.Bacc:
    nc = bacc.Bacc(target_bir_lowering=False)
    # x pre-tiled per 512-token chunk: [chunk, partition, t, tok] with 8KB
    # contiguous per partition row -> full-rate DMA.
    xH = nc.dram_tensor("xH", [8, 128, 8, 512], BF16, kind="ExternalInput")
    # weights per-g so the k/q parts arrive first: [g, partition, t, hd2]
    wH = nc.dram_tensor("wH", [3, 128, 8, 2 * HD], BF16, kind="ExternalInput")
    woutT = nc.dram_tensor("woutT", [2 * HD, D], BF16, kind="ExternalInput")
    out = nc.dram_tensor("out", [T, D], BF16, kind="ExternalOutput")
    # tail quarter ships per-head unnormalized projections + raw denominators;
    # the host divides (normalization commutes with the output projection)
    out2 = nc.dram_tensor("out2", [2, QW, D], BF16, kind="ExternalOutput")
    den = nc.dram_tensor("den", [2, QW], BF16, kind="ExternalOutput")
    if DEBUG_DENOM:
        dbg = nc.dram_tensor("dbg", [32, 512], F32, kind="ExternalOutput")

    with tile.TileContext(nc) as tc, ExitStack() as ctx:
        const = ctx.enter_context(tc.tile_pool(name="const", bufs=1))
        sb = ctx.enter_context(tc.tile_pool(name="sb", bufs=1))
        ps = ctx.enter_context(tc.tile_pool(name="ps", bufs=1, space="PSUM"))

        # ---- constants first (DVE memsets precede any vector-queue DMA) ----
        ones64 = const.tile([1, 64], BF16)
        nc.vector.memset(ones64, 1.0)
        wu_l = const.tile([128, 16], BF16)
        nc.vector.memset(wu_l, 0.5)
        wu_r = const.tile([128, 512], BF16)
        nc.vector.memset(wu_r, 0.5)
        dum = const.tile([1, 16], F32)

        # ---- DMA plan (only sync/gpsimd/scalar may issue DMAs) ----
        # sync:   x-ch0 t01/t45 pieces, then t0-3 halves of x b0 ch1-3
        # gpsimd: x-ch0 t23/t67 pieces, t4-7 halves of b0, all of b1
        # scalar: weights (done issuing well before the first real exp)
        wg = [
            const.tile([128, 8, 2 * HD], BF16, name=f"wg{g}") for g in range(3)
        ]
        x_sb = {}
        for b in range(B):
            for ch in range(4):
                x_sb[b, ch] = sb.tile(
                    [128, 8, 512], BF16, tag="x", bufs=8, name=f"x{b}{ch}"
                )
        # tiny warm-up DMAs: the DMA path has a multi-us cold-start; wake
        # each queue with a 16B/partition transfer before the real loads
        dwarm = const.tile([128, 3, 8], BF16, name="dwarm")
        nc.sync.dma_start(out=dwarm[:, 0, :], in_=xH[0][:, 0, 0:8])
        nc.gpsimd.dma_start(out=dwarm[:, 1, :], in_=xH[0][:, 1, 0:8])
        nc.scalar.dma_start(out=dwarm[:, 2, :], in_=xH[0][:, 2, 0:8])
        nc.sync.dma_start(out=x_sb[0, 0][:, 0:2, :], in_=xH[0][:, 0:2, :])
        nc.gpsimd.dma_start(out=x_sb[0, 0][:, 2:4, :], in_=xH[0][:, 2:4, :])
        nc.scalar.dma_start(out=wg[1], in_=wH[1])
        nc.scalar.dma_start(out=wg[0], in_=wH[0])
        nc.sync.dma_start(out=x_sb[0, 0][:, 4:6, :], in_=xH[0][:, 4:6, :])
        nc.gpsimd.dma_start(out=x_sb[0, 0][:, 6:8, :], in_=xH[0][:, 6:8, :])
        nc.scalar.dma_start(out=wg[2], in_=wH[2])
        wo = const.tile([2 * HD, D], BF16)
        nc.scalar.dma_start(out=wo, in_=woutT[:, :])
        for ch in range(1, 4):
            nc.sync.dma_start(out=x_sb[0, ch][:, 0:4, :], in_=xH[ch][:, 0:4, :])
            nc.gpsimd.dma_start(out=x_sb[0, ch][:, 4:8, :], in_=xH[ch][:, 4:8, :])
        for ch in range(4):
            nc.gpsimd.dma_start(out=x_sb[1, ch], in_=xH[4 + ch])

        # ---- warmup: keep PE continuously busy until x-ch0 lands (~13us)
        # so HAM un-throttles before the eager QKV chain; exp table load ----
        wk0 = ps.tile([128, 512], F32, tag="wk", bufs=2, name="warm")
        for _ in range(14):
            nc.tensor.matmul(wk0[0:16, :], wu_l[:], wu_r[:], start=True, stop=True)
        nc.scalar.activation(dum[:], wu_l[0:1, 0:16], EXP, scale=1.0)

        kT, qT, va, oT = {}, {}, {}, {}
        acc_sb, rec_row = {}, {}

        qk_wk, v_wk = {}, {}

        def emit_qk(b, g, ch, dst, csl, t_range, evac_splits=1):
            """g: 0=q 1=k. Accumulate w.T@x for t in t_range into the shared
            psum ring; evacuate to dst[:, csl] bf16 on the last step."""
            if t_range[0] == 0:
                qk_wk[b, g, ch] = ps.tile(
                    [128, 512], F32, tag="wk", bufs=2, name="qkps"
                )
            wk = qk_wk[b, g, ch]
            for t in t_range:
                nc.tensor.matmul(
                    wk[:],
                    wg[g][:, t, :],
                    x_sb[b, ch][:, t, :],
                    start=(t == 0),
                    stop=(t == 7),
                )
            if t_range[-1] == 7:
                c0 = csl.start
                w_ = 512 // evac_splits
                for i in range(evac_splits):
                    nc.vector.tensor_copy(
                        dst[:, c0 + i * w_ : c0 + (i + 1) * w_],
                        wk[:, i * w_ : (i + 1) * w_],
                    )

        def emit_v(b, ti, half):
            """v token-chunk ti (128 tokens), x-stationary: out [tok, hd] for
            both heads; half 0: matmuls t=0..3, half 1: t=4..7 + build va."""
            j = ti % 4
            ch = ti // 4
            if half == 0 and j == 0:
                v_wk[b, ch] = ps.tile([128, 512], F32, tag="wk", bufs=2, name="vps")
            wk = v_wk[b, ch]
            jsl = slice(j * 128, (j + 1) * 128)
            for t in range(4 * half, 4 * half + 4):
                nc.tensor.matmul(
                    wk[:, jsl],
                    x_sb[b, ch][:, t, j * 128 : (j + 1) * 128],
                    wg[2][:, t, :],
                    start=(t == 0),
                    stop=(t == 7),
                )
            if half == 1:
                vt = sb.tile([128, 130], BF16, tag="va", bufs=32, name=f"va{b}_{ti}")
                nc.vector.tensor_copy(vt[:, 0:64], wk[:, j * 128 : j * 128 + 64])
                nc.vector.tensor_copy(vt[:, 65:129], wk[:, j * 128 + 64 : (j + 1) * 128])
                nc.vector.memset(vt[:, 64:65], 1.0)
                nc.vector.memset(vt[:, 129:130], 1.0)
                va[b, ti] = vt

        for b in range(B):
            kT[b] = sb.tile([128, S], BF16, tag="kt", bufs=2, name=f"kT{b}")
            for Q in range(NQ):
                qT[b, Q] = sb.tile([128, QW], BF16, tag="qt", bufs=8, name=f"qT{b}{Q}")

        # ---- eager prefix: chunk-0 k, q, v(ti0,ti1) so attention can start
        # immediately; everything else goes through the worklist ----
        with nc.named_scope("head"):
            # k/q interleaved at t-pair grain so PE chases the x-ch0 pieces;
            # the eager v work moves to after the first scores (post0 below)
            emit_qk(0, 1, 0, kT[0], slice(0, 512), range(0, 2))
            emit_qk(0, 0, 0, qT[0, 0], slice(0, 512), range(0, 2))
            emit_qk(0, 1, 0, kT[0], slice(0, 512), range(2, 4))
            emit_qk(0, 0, 0, qT[0, 0], slice(0, 512), range(2, 4))
            emit_qk(0, 1, 0, kT[0], slice(0, 512), range(4, 6))
            emit_qk(0, 0, 0, qT[0, 0], slice(0, 512), range(4, 6))
            # kT evac in halves so scores ki0/ki1 start before the full cast
            emit_qk(0, 1, 0, kT[0], slice(0, 512), range(6, 8), evac_splits=2)
            emit_qk(0, 0, 0, qT[0, 0], slice(0, 512), range(6, 8))

        def post0():
            for ti in (0, 1):
                emit_v(0, ti, 0)
                emit_v(0, ti, 1)

        # ---- worklist: QKV series in deadline order ----
        wl = Worklist()

        def k_series(b, ch):
            csl = slice(ch * 512, (ch + 1) * 512)
            return [
                (lambda b=b, ch=ch, csl=csl, t0=t0: emit_qk(
                    b, 1, ch, kT[b], csl, range(t0, t0 + 2)
                ))
                for t0 in range(0, 8, 2)
            ]

        def q_series(b, Q):
            return [
                (lambda b=b, Q=Q, t0=t0: emit_qk(
                    b, 0, Q, qT[b, Q], slice(0, 512), range(t0, t0 + 2)
                ))
                for t0 in range(0, 8, 2)
            ]

        def v_series(b, ch):
            # 4 ti x 2 halves sharing one wk [128,512] psum tile
            fns = []
            for jj in range(4):
                ti = ch * 4 + jj
                fns.append(lambda b=b, ti=ti: emit_v(b, ti, 0))
                fns.append(lambda b=b, ti=ti: emit_v(b, ti, 1))
            return fns

        # batch 0: remaining chunks; deadlines from quarter-0's ki loop
        wl.add_a(k_series(0, 1), deadline=4)
        wl.add_a(v_series(0, 0)[4:], deadline=4)  # ti2, ti3 (ti0/1 eager)
        wl.add_a(v_series(0, 1), deadline=6)  # ti4-7 due AV slots 6..9
        wl.add_a(k_series(0, 2), deadline=8)
        wl.add_a(v_series(0, 2), deadline=10)
        wl.add_a(k_series(0, 3), deadline=12)
        wl.add_a(v_series(0, 3), deadline=14)
        wl.add_a(q_series(0, 1), deadline=15)
        wl.add_a(q_series(0, 2), deadline=31)
        wl.add_a(q_series(0, 3), deadline=47)
        # batch 1 (x lands ~9.5-19us =~ slots 2-10; paced pops reach these
        # around slot 25; ready guards for safety)
        wl.add_a(k_series(1, 0), ready=6, deadline=63)
        wl.add_a(q_series(1, 0), ready=6, deadline=63)
        wl.add_a(v_series(1, 0), ready=6, deadline=66)
        wl.add_a(k_series(1, 1), ready=6, deadline=68)
        wl.add_a(v_series(1, 1), ready=6, deadline=70)
        wl.add_a(k_series(1, 2), ready=8, deadline=72)
        wl.add_a(v_series(1, 2), ready=8, deadline=74)
        wl.add_a(k_series(1, 3), ready=10, deadline=76)
        wl.add_a(v_series(1, 3), ready=10, deadline=78)
        wl.add_a(q_series(1, 1), ready=10, deadline=79)
        wl.add_a(q_series(1, 2), ready=10, deadline=95)
        wl.add_a(q_series(1, 3), ready=10, deadline=111)

        def norm_closure(b, Q, h):
            def f():
                # broadcast 1/denom across 64 partitions (K=1 matmul), then
                # normalize into oT rows for this head (partition-shifted).
                bc = ps.tile([64, 512], F32, tag="wk", bufs=2, name="bc")
                nc.tensor.matmul(
                    bc[:], ones64[:], rec_row[b, Q, h], start=True, stop=True
                )
                p0 = h * 64
                nc.vector.tensor_mul(
                    oT[b, Q][p0 : p0 + 64, :], acc_sb[b, Q, h][0:64, :], bc[:]
                )
            return f

        def op_closure(b, Q, tc_i, nk):
            def f():
                ob = sb.tile([128, 512], BF16, tag="ob", bufs=8, name="ob")
                tsl = slice(tc_i * 128, (tc_i + 1) * 128)
                op = ps.tile([128, 512], F32, tag="wk", bufs=2, name="opps")
                nc.tensor.matmul(
                    op[:],
                    oT[b, Q][:, tsl],
                    wo[:, nk * 512 : (nk + 1) * 512],
                    start=True,
                    stop=True,
                )
                nc.vector.tensor_copy(ob[:], op[:])
                r0 = b * S + (Q * 4 + tc_i) * 128
                nc.sync.dma_start(
                    out=out[r0 : r0 + 128, nk * 512 : (nk + 1) * 512], in_=ob[:]
                )
            return f

        def attn_quarter(b, Q, wl, slot_base, post_first=None):
            accs = [
                ps.tile([65, 512], F32, tag="acc", bufs=2, name=f"ac{b}{Q}{h}")
                for h in range(2)
            ]
            prs = {}

            def emit_av(ki):
                for h in range(2):
                    nc.tensor.matmul(
                        accs[h][:],
                        va[b, ki][:, h * 65 : (h + 1) * 65],
                        prs[ki][:, h * 512 : (h + 1) * 512],
                        start=(ki == 0),
                        stop=(ki == 15),
                    )

            def emit_scores(ki):
                sc = ps.tile([128, 1024], F32, tag="sc", bufs=2, name="sc")
                ksl = slice(ki * 128, (ki + 1) * 128)
                for h in range(2):
                    p0 = h * 64
                    nc.tensor.matmul(
                        sc[:, h * 512 : (h + 1) * 512],
                        kT[b][p0 : p0 + 64, ksl],
                        qT[b, Q][p0 : p0 + 64, :],
                        start=True,
                        stop=True,
                    )
                return sc

            # scores run one ki ahead of the exp stream so they are the
            # FIRST PE work of each slot — closure pops can no longer delay
            # the next exp by a whole pop burst
            scs = {0: emit_scores(0)}
            for ki in range(KI):
                pr = sb.tile([128, 1024], BF16, tag="pr", bufs=4, name="pr")
                nc.scalar.activation(pr[:], scs.pop(ki)[:], EXP, scale=SCALE)
                prs[ki] = pr
                if ki + 1 < KI:
                    scs[ki + 1] = emit_scores(ki + 1)
                if ki == 0 and post_first is not None:
                    post_first()
                if ki >= 2:
                    emit_av(ki - 2)
                wl.pop_for_slot(slot_base + ki)
            emit_av(KI - 2)
            emit_av(KI - 1)

            tail = b == 1 and Q == NQ - 1
            end_slot = slot_base + KI
            if tail:
                # host-normalization path: evacuate both heads' unnormalized
                # accumulators into one [128,512] bf16 tile (head h at
                # partitions 64h..64h+63), DMA the raw denominator rows out,
                # then per-head K=64 output projections (row-tiled pairs run
                # concurrently) shipped unnormalized; the host divides.
                # cast each head's accumulator in place (no partition shift),
                # then DMA h1's rows up to partitions 64-127 so the per-head
                # K=64 projections can run as a concurrent row-tiled pair
                ahs = []
                for h in range(2):
                    a = sb.tile([65, QW], BF16, tag="accsb", bufs=16, name="abf")
                    nc.vector.tensor_copy(a[:], accs[h][:])
                    nc.sync.dma_start(out=den[h : h + 1, :], in_=a[64:65, :])
                    ahs.append(a)
                abf = sb.tile([128, QW], BF16, tag="ot", bufs=8, name="abf2")
                nc.gpsimd.dma_start(out=abf[64:128, :], in_=ahs[1][0:64, :])

                def op2_closure(j, tc_i, nk):
                    def f():
                        # both heads as a concurrent row-tiled pair (K=64 at
                        # rows 0-63 / 64-127). Even units use one [128,1024]
                        # sc tile (2 banks) evacuated by ACT; odd units use a
                        # pair of [128,512] wk tiles evacuated by DVE — both
                        # psum rings and both evac engines stay busy.
                        tsl = slice(tc_i * 128, (tc_i + 1) * 128)
                        nksl = slice(nk * 512, (nk + 1) * 512)
                        if j & 1 == 0:
                            o2 = ps.tile([128, 1024], F32, tag="sc", bufs=2, name="o2ps")
                            oA, oB = o2[:, 0:512], o2[:, 512:1024]
                        else:
                            pA = ps.tile([128, 512], F32, tag="wk", bufs=2, name="o2a")
                            pB = ps.tile([128, 512], F32, tag="wk", bufs=2, name="o2b")
                            oA, oB = pA[:], pB[:]
                        nc.tensor.matmul(
                            oA, ahs[0][0:64, tsl], wo[0:64, nksl],
                            start=True, stop=True,
                        )
                        nc.tensor.matmul(
                            oB, abf[64:128, tsl], wo[64:128, nksl],
                            start=True, stop=True,
                        )
                        ob2 = sb.tile([128, 1024], BF16, tag="ob2", bufs=4, name="ob2")
                        if j & 1 == 0:
                            nc.scalar.copy(ob2[:], o2[:])
                        else:
                            nc.vector.tensor_copy(ob2[:, 0:512], oA)
                            nc.vector.tensor_copy(ob2[:, 512:1024], oB)
                        r0 = tc_i * 128
                        nc.sync.dma_start(
                            out=out2[0, r0 : r0 + 128, nksl], in_=ob2[:, 0:512]
                        )
                        nc.gpsimd.dma_start(
                            out=out2[1, r0 : r0 + 128, nksl], in_=ob2[:, 512:1024]
                        )
                    return f

                for j, (tc_i, nk) in enumerate(
                    (t, n) for t in range(4) for n in range(2)
                ):
                    wl.add_b([op2_closure(j, tc_i, nk)], ready=end_slot)
                return

            # quarter end: evacuate accumulators; 1/denominator via a DMA
            # round trip: gather the two [1,512] denominator rows into a
            # [128,8] tile (cheap cross-partition transpose on an idle DMA
            # engine), one tiny DVE reciprocal, scatter back to [1,512] rows.
            oT[b, Q] = sb.tile([128, QW], BF16, tag="ot", bufs=8, name=f"oT{b}{Q}")
            for h in range(2):
                a = sb.tile([65, 512], F32, tag="accsb", bufs=16, name="accsb")
                nc.vector.tensor_copy(a[:], accs[h][:])
                acc_sb[b, Q, h] = a
            dstage = sb.tile([128, 8], F32, tag="dstage", bufs=4, name="dstage")
            rstage = sb.tile([128, 8], BF16, tag="rstage", bufs=4, name="rstage")
            for h in range(2):
                nc.sync.dma_start(
                    out=dstage[:, h * 4 : (h + 1) * 4],
                    in_=acc_sb[b, Q, h][64:65, :],
                )
                with nc.allow_low_precision(reason="bf16 1/denom, ~0.4% is fine"):
                    nc.vector.reciprocal(
                        rstage[:, h * 4 : (h + 1) * 4], dstage[:, h * 4 : (h + 1) * 4]
                    )
                r = sb.tile([1, 512], BF16, tag="rec", bufs=16, name="rec")
                nc.sync.dma_start(out=r[:], in_=rstage[:, h * 4 : (h + 1) * 4])
                rec_row[b, Q, h] = r[:]
            # norm + projection closures are deferred into the batch-1 era
            # (slots 64+) where the PE has slack; batch-0's era is already
            # oversubscribed building batch-1's QKV. Stagger per quarter so
            # the deferred work doesn't flood DVE all at once.
            rdy = max(end_slot + 3, 66 + 5 * Q if b == 0 else 0)
            for h in range(2):
                wl.add_b([norm_closure(b, Q, h)], ready=rdy)
            for tc_i in range(4):
                for nk in range(2):
                    wl.add_b([op_closure(b, Q, tc_i, nk)], ready=rdy + 1)

        with nc.named_scope("attn0"):
            for Q in range(NQ):
                attn_quarter(0, Q, wl, Q * KI, post_first=post0 if Q == 0 else None)
        with nc.named_scope("attn1"):
            for Q in range(NQ):
                attn_quarter(1, Q, wl, 64 + Q * KI)
        with nc.named_scope("tail"):
            wl.drain()

    nc.finalize()
    return nc


_NC_CACHE = None
TRACE = False  # set True (e.g. from test.py) to capture an NTFF profile
LAST_RESULT = None  # BassKernelResults of the most recent run


def _get_nc():
    global _NC_CACHE
    if _NC_CACHE is None:
        _NC_CACHE = build_kernel()
    return _NC_CACHE


def kernel(x, W_qkv, W_out, b_out):
    import ml_dtypes

    x = np.asarray(x, dtype=np.float32)
    W_qkv = np.asarray(W_qkv, dtype=np.float32)
    W_out = np.asarray(W_out, dtype=np.float32)
    b_out = np.asarray(b_out, dtype=np.float32)

    # [chunk, partition, t, tok]: per-partition-contiguous chunks for
    # full-rate DMA on device
    xf = x.reshape(T, D)  # [tok, d]
    xH = np.ascontiguousarray(
        xf.reshape(8, 512, 8, 128).transpose(0, 3, 2, 1)
    ).astype(ml_dtypes.bfloat16)
    in_maps = []
    for c in range(N_CORES):
        h0 = c * HEADS_PER_CORE
        rows = slice(h0 * HD, (h0 + 2) * HD)  # this core's 128 head dims
        wq = W_qkv[0 * D :][rows]  # [128, D]
        wk = W_qkv[1 * D :][rows]
        wv = W_qkv[2 * D :][rows]
        # [g, partition, t, hd2]: per-g transposed weight chunks
        wH = np.ascontiguousarray(
            np.stack([w.T.reshape(8, 128, 2 * HD) for w in (wq, wk, wv)]).transpose(
                0, 2, 1, 3
            )
        ).astype(ml_dtypes.bfloat16)
        woutT = np.ascontiguousarray(W_out[:, h0 * HD : (h0 + 2) * HD].T).astype(
            ml_dtypes.bfloat16
        )
        in_maps.append({"xH": xH, "wH": wH, "woutT": woutT})

    nc = _get_nc()
    global LAST_RESULT
    res = run_bass_kernel_spmd(nc, in_maps, core_ids=list(range(N_CORES)), trace=TRACE)
    LAST_RESULT = res
    partial = np.zeros((T, D), dtype=np.float64)
    for c in range(N_CORES):
        r = res.results[c]
        partial[: T - QW] += r["out"][: T - QW].astype(np.float64)
        # tail quarter: per-head unnormalized projections / raw denominators
        for h in range(HEADS_PER_CORE):
            partial[T - QW :] += r["out2"][h].astype(np.float64) / r["den"][h].astype(
                np.float64
            )[:, None]
    full = (partial + b_out.astype(np.float64)).astype(np.float32)
    return full.reshape(B, S, D)
